# revision 51
# baseline (speedup 1.0000x reference)
"""Multi-head attention (B=2, S=2048, D=1024, H=16, causal + rel-pos-bias + RoPE)
on 8 Trainium2 NeuronCores.

Sharding: core c handles batch c//4 and head-group c%4 (4 heads = 256 model dims).
Each core computes its heads' Q/K/V projections (column-sharded weights), RoPE,
causal attention with relative position bias, and a partial output projection
(row-sharded Wo). Host sums the 4 fp16 partials per batch and adds Wo_b.

v2 design notes (vs baseline):
- RoPE rotate-half swap via SBUF->SBUF DMAs instead of ScalarE copies.
- Softmax denominators extracted with ScalarE shifted copies into one tile at
  partitions {0,32,64,96}, one batched reciprocal per q-chunk, and the
  1/l broadcast materialized by a stride-0 DMA; normalization is one in-place
  bf16 multiply per (qc, m).
- exp(bias) (with causal mask folded in) multiplies praw in one wide in-place
  bf16 tensor_tensor per quad.
- fp16 output partials (half the store traffic).
- attention emitted right after Q/K projections so ScalarE exp (the attention
  bottleneck) starts as early as possible.
"""

import math

import numpy as np
import ml_dtypes

import concourse.bass as bass
import concourse.mybir as mybir
import concourse.tile as tile
from concourse import bacc
from concourse.bass_utils import run_bass_kernel_spmd

BF16 = ml_dtypes.bfloat16

B, S, D, H = 2, 2048, 1024, 16
DK = 64
SCALE = math.sqrt(DK)
HPC = 4          # heads per core
GDIM = HPC * DK  # 256 model dims per core
N_CORES = 8
KT = S // 128    # 16 k-tiles
QC = S // 512    # 4 q-chunks

f32 = mybir.dt.float32
bf16 = mybir.dt.bfloat16
fp16 = mybir.dt.float16


def _quads(qc):
    """kt quad-groups for one (h, qc) chunk: list of [(kt, n, q0), ...] (4 kts each)."""
    kts = list(range(4 * qc + 4))
    out = []
    for i in range(0, len(kts), 4):
        grp = []
        for kt in kts[i:i + 4]:
            if kt // 4 == qc:
                n = 512 - 128 * (kt % 4)
                q0 = 128 * kt
            else:
                n = 512
                q0 = 512 * qc
            grp.append((kt, n, q0))
        out.append(grp)
    return out


# eb elements: per (qc, m, quad): 128 * 2 * gn
EB_TOTAL = sum(128 * 2 * sum(n for kt, n, q0 in grp)
               for qc in range(QC) for m in range(2) for grp in _quads(qc))

_PROGRAM = None


def _build_program():
    nc = bacc.Bacc("TRN2", target_bir_lowering=False, debug=False)

    dqT = nc.dram_tensor("qT", [QC, 128, 8, 512], bf16, kind="ExternalInput").ap()
    dkT = nc.dram_tensor("kT", [QC, 128, 8, 512], bf16, kind="ExternalInput").ap()
    dvT = nc.dram_tensor("vT", [QC, 128, 8, 512], bf16, kind="ExternalInput").ap()
    dwq = nc.dram_tensor("wq", [8, 128, GDIM], bf16, kind="ExternalInput").ap()
    dwk = nc.dram_tensor("wk", [8, 128, GDIM], bf16, kind="ExternalInput").ap()
    dwv = nc.dram_tensor("wv", [8, 128, GDIM], bf16, kind="ExternalInput").ap()
    dwo = nc.dram_tensor("wo", [2, 128, D], bf16, kind="ExternalInput").ap()
    deb = nc.dram_tensor("eb", [EB_TOTAL], bf16, kind="ExternalInput").ap()
    dcos = nc.dram_tensor("cosT", [128, S], bf16, kind="ExternalInput").ap()
    dsin = nc.dram_tensor("sinT", [128, S], bf16, kind="ExternalInput").ap()
    dout = nc.dram_tensor("out", [S, D], fp16, kind="ExternalOutput").ap()

    with tile.TileContext(nc) as tc:
        with tc.tile_pool(name="consts", bufs=1) as consts, \
             tc.tile_pool(name="persist", bufs=1) as persist, \
             tc.tile_pool(name="xstage", bufs=2) as xstage, \
             tc.tile_pool(name="ropep", bufs=2) as ropep, \
             tc.tile_pool(name="prawp", bufs=3) as prawp, \
             tc.tile_pool(name="ebp", bufs=4) as ebp, \
             tc.tile_pool(name="normp", bufs=2) as normp, \
             tc.tile_pool(name="outst", bufs=3) as outst, \
             tc.tile_pool(name="psum_big", bufs=3, space="PSUM") as psb, \
             tc.tile_pool(name="psum_cx", bufs=2, space="PSUM") as psc:

            # ---- constants (scalar/ACT HWDGE queue: pure prefetch, phase 1) ----
            wq_s = consts.tile([128, 8, GDIM], bf16)
            wk_s = consts.tile([128, 8, GDIM], bf16)
            wv_s = consts.tile([128, 8, GDIM], bf16)
            wo_s = consts.tile([128, 2, D], bf16)
            cos_s = consts.tile([128, S], bf16)
            sin_s = consts.tile([128, S], bf16)
            # single DMA per weight tensor: DRAM-side AP reorder makes the
            # partition-major SBUF layout legal in one transfer (28 -> 6 DMA
            # instructions; each occupies the ACT FIFO ~0.6us)
            nc.scalar.dma_start(out=wk_s, in_=dwk.rearrange("t p n -> p t n"))
            nc.scalar.dma_start(out=wq_s, in_=dwq.rearrange("t p n -> p t n"))
            nc.scalar.dma_start(out=cos_s, in_=dcos)
            nc.scalar.dma_start(out=sin_s, in_=dsin)
            nc.scalar.dma_start(out=wv_s, in_=dwv.rearrange("t p n -> p t n"))
            nc.scalar.dma_start(out=wo_s, in_=dwo.rearrange("t p n -> p t n"))

            # preload the exp table set during the idle prologue window
            # (ACT FIFO is clear after the 6 const DMA issues)
            warm = consts.tile([1, 16], f32)
            nc.scalar.activation(out=warm, in_=wk_s[0:1, 0, 0:16],
                                 func=mybir.ActivationFunctionType.Exp)

            # ---- persistent activations ----
            QT = [persist.tile([128, S], bf16, name=f"QT{m}") for m in range(2)]
            KTt = [persist.tile([128, S], bf16, name=f"KTt{m}") for m in range(2)]
            U = [persist.tile([128, S], bf16, name=f"U{m}") for m in range(2)]
            Vt = persist.tile([128, KT, HPC, DK + 1], bf16)
            nc.vector.memset(Vt[:, :, :, DK:DK + 1], 1.0)

            def load_xchunk(dsrc, which, c):
                xc = xstage.tile([128, 8, 512], bf16, tag=f"x{which}",
                                 name=f"x{which}{c}")
                nc.sync.dma_start(out=xc, in_=dsrc[c])
                return xc

            def project_rope_chunk(xc, wsrc, dst, c, which, ms=(0, 1)):
                """One 512-token chunk of the Q/K projection + rope."""
                for m in ms:
                    pp = psb.tile([128, 512], f32, tag="big",
                                  name=f"pp{which}{m}{c}")
                    for t in range(8):
                        nc.tensor.matmul(
                            pp,
                            lhsT=wsrc[:, t, 128 * m:128 * m + 128],
                            rhs=xc[:, t, :],
                            start=(t == 0), stop=(t == 7))
                    qraw = ropep.tile([128, 512], bf16, tag="qraw")
                    nc.vector.tensor_copy(out=qraw, in_=pp)
                    # rotate-half swap via sbuf->sbuf DMAs; sin sign-folded.
                    sw = ropep.tile([128, 512], bf16, tag="sw")
                    for base in (0, 64):
                        nc.vector.tensor_copy(out=sw[base:base + 32],
                                              in_=qraw[base + 32:base + 64])
                        nc.vector.tensor_copy(out=sw[base + 32:base + 64],
                                              in_=qraw[base:base + 32])
                    cc = ropep.tile([128, 512], bf16, tag="cc")
                    nc.vector.tensor_mul(out=cc, in0=qraw,
                                         in1=cos_s[:, 512 * c:512 * c + 512])
                    nc.vector.tensor_mul(out=sw, in0=sw,
                                         in1=sin_s[:, 512 * c:512 * c + 512])
                    nc.vector.tensor_add(out=dst[m][:, 512 * c:512 * c + 512],
                                         in0=cc, in1=sw)

            def project_v_chunk(xc, c, halves=(0, 1)):
                for half in halves:
                    pv = psb.tile([128, 2, GDIM], f32, tag="big",
                                  name=f"pv{c}{half}")
                    for jj in range(2):
                        j = 2 * half + jj
                        for t in range(8):
                            nc.tensor.matmul(
                                pv[:, jj, :],
                                lhsT=xc[:, t, 128 * j:128 * j + 128],
                                rhs=wv_s[:, t, :],
                                start=(t == 0), stop=(t == 7))
                    kt0 = 4 * c + 2 * half
                    nc.scalar.copy(
                        out=Vt[:, kt0:kt0 + 2, :, 0:DK],
                        in_=pv.rearrange("p j (h d) -> p j h d", h=HPC))

            # outproj(qc) is deferred into qc+1's stage so the PE stream never
            # head-of-line blocks on the norm chain (recip/bcast DMA latency).
            def outproj(qc, out_eng=None):
                ost4 = outst.tile([128, 4, D], fp16, tag="ost", bufs=2)
                for i, tt in enumerate(range(4 * qc, 4 * qc + 4)):
                    po = [psc.tile([128, 512], f32, tag="pcx",
                                   name=f"po{tt}{e}") for e in range(2)]
                    for m in range(2):
                        for e in range(2):
                            nc.tensor.matmul(
                                po[e],
                                lhsT=U[m][:, 128 * tt:128 * tt + 128],
                                rhs=wo_s[:, m, 512 * e:512 * e + 512],
                                start=(m == 0), stop=(m == 1))
                    nc.scalar.copy(out=ost4[:, i, 0:512], in_=po[0])
                    nc.vector.tensor_copy(out=ost4[:, i, 512:1024], in_=po[1])
                # one 1MB store per stage: fewer FIFO slots + completion receipts
                (out_eng or nc.gpsimd).dma_start(
                    out=dout[512 * qc:512 * qc + 512, :].rearrange(
                        "(tt p) n -> p tt n", tt=4),
                    in_=ost4)

            eb_off = [0]

            def attention_quad(qc, m, grp, pcx):
                for _once in range(1):
                    last_kt = 4 * qc + 3
                    gn = sum(n for kt, n, q0 in grp)
                    praw = prawp.tile([128, 2, gn], bf16, tag="praw")
                    ebt = ebp.tile([128, 2, gn], bf16, tag="ebt")
                    eb_sz = 128 * 2 * gn
                    nc.sync.dma_start(
                        out=ebt,
                        in_=deb[eb_off[0]:eb_off[0] + eb_sz].rearrange(
                            "(p n) -> p n", p=128))
                    eb_off[0] += eb_sz
                    goff = 0
                    for pi in range(0, 4, 2):
                        pair = grp[pi:pi + 2]
                        pn = sum(n for kt, n, q0 in pair)
                        pss = [psb.tile([128, pn], f32, tag="big",
                                        name=f"ps{qc}{m}{pi}{a}") for a in range(2)]
                        for a in range(2):
                            soff = 0
                            for kt, n, q0 in pair:
                                nc.tensor.matmul(
                                    pss[a][:, soff:soff + n],
                                    lhsT=KTt[m][64 * a:64 * a + DK,
                                                128 * kt:128 * kt + 128],
                                    rhs=QT[m][64 * a:64 * a + DK, q0:q0 + n],
                                    start=True, stop=True,
                                    tile_position=(64 * a, 0))
                                soff += n
                        for a in range(2):
                            nc.scalar.activation(
                                out=praw[:, a, goff:goff + pn], in_=pss[a],
                                func=mybir.ActivationFunctionType.Exp)
                        # bias (+ causal mask) multiply per pair (short chain)
                        nc.vector.tensor_mul(
                            out=praw[:, :, goff:goff + pn],
                            in0=praw[:, :, goff:goff + pn],
                            in1=ebt[:, :, goff:goff + pn])
                        soff = 0
                        for kt, n, q0 in pair:
                            co = q0 - 512 * qc
                            for a in range(2):
                                nc.tensor.matmul(
                                    pcx[a][0:DK + 1, co:co + n],
                                    lhsT=Vt[:, kt, 2 * m + a, :],
                                    rhs=praw[:, a, goff + soff:goff + soff + n],
                                    start=(kt == 0), stop=(kt == last_kt))
                            soff += n
                        goff += pn
            def norm_group(qc, m, pcx):
                # extract unnormalized ctx + denominators into SBUF staging
                lst = normp.tile([64, 512], f32, tag="lst", bufs=2,
                                 name=f"lst{qc}{m}")
                ucx = [None, None]
                for a in range(2):
                    ucx[a] = normp.tile([DK + 1, 512], bf16, tag="ucx",
                                        bufs=5, name=f"ucx{qc}{m}{a}")
                    nc.scalar.copy(out=ucx[a], in_=pcx[a][0:DK + 1, :])
                    nc.vector.tensor_copy(out=lst[32 * a:32 * a + 1, :],
                                          in_=ucx[a][DK:DK + 1, :])
                linv = normp.tile([64, 512], f32, tag="linv", bufs=2,
                                  name=f"linv{qc}{m}")
                nc.vector.reciprocal_approx_fast(out=linv, in_=lst)
                linvb = normp.tile([64, 512], bf16, tag="linvb", bufs=2,
                                   name=f"linvb{qc}{m}")
                nc.vector.tensor_copy(out=linvb, in_=linv)
                bceng = nc.sync if qc == QC - 1 else nc.gpsimd
                for a in range(2):
                    bcm = normp.tile([64, 512], bf16, tag="bcm", bufs=4,
                                     name=f"bcm{qc}{m}{a}")
                    bceng.dma_start(
                        out=bcm,
                        in_=linvb[32 * a:32 * a + 1, None, :].to_broadcast(
                            (1, 64, 512)))
                    nc.vector.tensor_mul(
                        out=U[m][64 * a:64 * a + DK, 512 * qc:512 * qc + 512],
                        in0=ucx[a][0:DK, :],
                        in1=bcm)

            # ---- staged pipeline: proj chunk qc+1 emitted mid-stage qc so
            # the rope chain is never behind the norm/ost work in the DVE FIFO
            xk = load_xchunk(dkT, "k", 0)
            xq = load_xchunk(dqT, "q", 0)
            xv = load_xchunk(dvT, "v", 0)
            project_rope_chunk(xk, wk_s, KTt, 0, "k")
            project_rope_chunk(xq, wq_s, QT, 0, "q")
            project_v_chunk(xv, 0)
            for qc in range(QC):
                quads = _quads(qc)
                pcx = [psc.tile([128, 512], f32, tag="pcx",
                                name=f"pcx{qc}0{a}") for a in range(2)]
                for grp in quads:
                    attention_quad(qc, 0, grp, pcx)
                norm_group(qc, 0, pcx)
                if qc >= 1:
                    outproj(qc - 1)
                pcx = [psc.tile([128, 512], f32, tag="pcx",
                                name=f"pcx{qc}1{a}") for a in range(2)]

                def proj_step(which):
                    # each x load slots between m1 ebt DMAs on the sync FIFO;
                    # the proj matmuls fill ACT-bound gaps in the m1 group
                    if which == "k":
                        project_rope_chunk(load_xchunk(dkT, "k", qc + 1),
                                           wk_s, KTt, qc + 1, "k")
                    elif which == "q":
                        project_rope_chunk(load_xchunk(dqT, "q", qc + 1),
                                           wq_s, QT, qc + 1, "q")
                    else:
                        project_v_chunk(load_xchunk(dvT, "v", qc + 1), qc + 1)

                projs = ["k", "q", "v"] if qc + 1 < QC else []
                for gi, grp in enumerate(quads):
                    attention_quad(qc, 1, grp, pcx)
                    if gi < len(projs):
                        proj_step(projs[gi])
                for which in projs[len(quads):]:
                    proj_step(which)
                norm_group(qc, 1, pcx)
            outproj(QC - 1, out_eng=nc.sync)
            assert eb_off[0] == EB_TOTAL

    nc.compile()
    return nc


def _get_program():
    global _PROGRAM
    if _PROGRAM is None:
        _PROGRAM = _build_program()
    return _PROGRAM


def _rope_tables():
    half = DK // 2
    inv_freq = 1.0 / (10000.0 ** (np.arange(half, dtype=np.float64) / half))
    ang = np.arange(S, dtype=np.float64)[:, None] * inv_freq[None, :]  # [S, 32]
    cos = np.cos(ang).T  # [32, S]
    sin = np.sin(ang).T
    cos64 = np.concatenate([cos, cos], axis=0)            # [64, S]
    sin64 = np.concatenate([-sin, sin], axis=0)           # signed for rotate-half
    cosT = np.tile(cos64, (2, 1)).astype(BF16)            # [128, S]
    sinT = np.tile(sin64, (2, 1)).astype(BF16)
    return np.ascontiguousarray(cosT), np.ascontiguousarray(sinT)


def _pack_ebias(bias_g):
    """bias_g: [HPC, S, S] f32 (this group's heads). Returns packed 1D bf16
    log-domain additive bias (masked entries = -30000): per (qc, m, quad) one
    contiguous [128, 2, gn] block (a interleaved)."""
    out = np.empty(EB_TOTAL, dtype=BF16)
    off = 0
    for qc in range(QC):
        for m in range(2):
            for grp in _quads(qc):
                blks = []  # per a: [128, gn]
                for a in range(2):
                    h = 2 * m + a
                    cols = []
                    for kt, n, q0 in grp:
                        blk = np.exp(
                            bias_g[h, q0:q0 + n, 128 * kt:128 * kt + 128]
                            .astype(np.float64)).T.astype(np.float32)  # [128, n]
                        if kt // 4 == qc:
                            blk[:, 0:128] *= np.triu(
                                np.ones((128, 128), dtype=np.float32))
                        cols.append(blk)
                    blks.append(np.concatenate(cols, axis=1))
                wide = np.stack(blks, axis=1)  # [128, 2, gn]
                sz = wide.size
                out[off:off + sz] = wide.astype(BF16).reshape(-1)
                off += sz
    assert off == EB_TOTAL
    return out


def _prep_inputs(query, key, value, rel_pos_bias, Wq, Wk, Wv, Wo_w):
    cosT, sinT = _rope_tables()
    xT = {}
    for nm, x in (("q", query), ("k", key), ("v", value)):
        for b in range(B):
            t = np.ascontiguousarray(
                x[b].T.reshape(8, 128, QC, 512).transpose(2, 1, 0, 3)
            ).astype(BF16)
            xT[(nm, b)] = t
    wqs, wks, wvs, wos, ebs = {}, {}, {}, {}, {}
    for g in range(4):
        sl = slice(GDIM * g, GDIM * (g + 1))
        wqs[g] = np.ascontiguousarray(
            (Wq[sl, :] / SCALE).T.reshape(8, 128, GDIM)).astype(BF16)
        wks[g] = np.ascontiguousarray(Wk[sl, :].T.reshape(8, 128, GDIM)).astype(BF16)
        wvs[g] = np.ascontiguousarray(Wv[sl, :].T.reshape(8, 128, GDIM)).astype(BF16)
        wos[g] = np.ascontiguousarray(Wo_w[:, sl].T.reshape(2, 128, D)).astype(BF16)
        ebs[g] = _pack_ebias(rel_pos_bias[0, HPC * g:HPC * (g + 1)])
    in_maps = []
    for c in range(N_CORES):
        b, g = c // 4, c % 4
        in_maps.append({
            "qT": xT[("q", b)], "kT": xT[("k", b)], "vT": xT[("v", b)],
            "wq": wqs[g], "wk": wks[g], "wv": wvs[g], "wo": wos[g],
            "eb": ebs[g], "cosT": cosT, "sinT": sinT,
        })
    return in_maps


def _run(query, key, value, rel_pos_bias, Wq, Wk, Wv, Wo_w, Wo_b, trace=False,
         **trace_kwargs):
    nc = _get_program()
    in_maps = _prep_inputs(query, key, value, rel_pos_bias, Wq, Wk, Wv, Wo_w)
    res = run_bass_kernel_spmd(nc, in_maps, core_ids=list(range(N_CORES)),
                               trace=trace, **trace_kwargs)
    out = np.empty((B, S, D), dtype=np.float32)
    for b in range(B):
        acc = res.results[4 * b]["out"].astype(np.float32)
        for g in range(1, 4):
            acc = acc + res.results[4 * b + g]["out"].astype(np.float32)
        out[b] = acc + Wo_b[None, :]
    return out, res


def _cpu_fallback(query, key, value, mask, rel_pos_bias, Wq, Wk, Wv, Wo_w, Wo_b):
    def rope_np(x):
        half = DK // 2
        inv_freq = 1.0 / (10000.0 ** (np.arange(half, dtype=np.float32) / half))
        ang = np.arange(S, dtype=np.float32)[:, None] * inv_freq[None, :]
        cos = np.concatenate([np.cos(ang), np.cos(ang)], axis=-1)[None, None]
        sin = np.concatenate([np.sin(ang), np.sin(ang)], axis=-1)[None, None]
        x1, x2 = x[..., :half], x[..., half:]
        rot = np.concatenate([-x2, x1], axis=-1)
        return x * cos + rot * sin

    q = np.einsum('bsd,ed->bse', query, Wq).reshape(B, S, H, DK).transpose(0, 2, 1, 3)
    k = np.einsum('bsd,ed->bse', key, Wk).reshape(B, S, H, DK).transpose(0, 2, 1, 3)
    v = np.einsum('bsd,ed->bse', value, Wv).reshape(B, S, H, DK).transpose(0, 2, 1, 3)
    q, k = rope_np(q), rope_np(k)
    sc = np.einsum('bhqd,bhkd->bhqk', q, k) / SCALE + rel_pos_bias
    sc = np.where(mask, sc, -np.inf)
    sc = sc - sc.max(axis=-1, keepdims=True)
    e = np.exp(sc)
    attn = e / e.sum(axis=-1, keepdims=True)
    ctx = np.einsum('bhqk,bhkd->bhqd', attn, v)
    ctx = ctx.transpose(0, 2, 1, 3).reshape(B, S, D)
    return (np.einsum('bsd,ed->bse', ctx, Wo_w) + Wo_b).astype(np.float32)


def kernel(query, key, value, mask, rel_pos_bias, Wq, Wk, Wv, Wo_w, Wo_b):
    query = np.asarray(query, dtype=np.float32)
    key = np.asarray(key, dtype=np.float32)
    value = np.asarray(value, dtype=np.float32)
    mask = np.asarray(mask)
    rel_pos_bias = np.asarray(rel_pos_bias, dtype=np.float32)
    Wq = np.asarray(Wq, dtype=np.float32)
    Wk = np.asarray(Wk, dtype=np.float32)
    Wv = np.asarray(Wv, dtype=np.float32)
    Wo_w = np.asarray(Wo_w, dtype=np.float32)
    Wo_b = np.asarray(Wo_b, dtype=np.float32)

    if not np.array_equal(mask.reshape(S, S),
                          np.tril(np.ones((S, S), dtype=bool))):
        return _cpu_fallback(query, key, value, mask, rel_pos_bias,
                             Wq, Wk, Wv, Wo_w, Wo_b)

    out, _ = _run(query, key, value, rel_pos_bias, Wq, Wk, Wv, Wo_w, Wo_b)
    return out
